# revision 22
# baseline (speedup 1.0000x reference)
"""Trainium2 Bass kernel for nn_Agg_57380763075323 (segment_reduce).

Computes, for each (batch, span): [min, max, mean] over the span's tokens of
x[B=16, T=8192, D=256], output [B, S=512, 3*D=768] float32.

Device fast path assumes the uniform span structure produced by
setup_inputs(): span s covers tokens [s*16, (s+1)*16) for all examples.
Anything else falls back to an exact numpy implementation of the reference
semantics (searchsorted-based segment assignment).

Sharding: data-parallel over batch; each of the 8 NeuronCores processes 2
examples. No cross-core communication.

Precision strategy: the output tolerance is rel_err < 2e-2; x is converted
to fp16 on the host during the shard step (rel err <= 2^-11 ~ 5e-4).  This
halves HBM load traffic (the memory-bound roofline) and doubles DVE
tensor_tensor throughput (2x_1P perf mode for 16-bit dtypes).

Device algorithm per core (2 examples, each [8192, 256] fp16), working in
PAIRS of 128-span tiles ([128, 2, 4096] fp16 = one 2MB DMA):
  - min and max via pairwise log-trees of fp16 tensor_tensor ops on the
    Vector engine; each tree level is ONE batched op across the pair
    (3D access patterns), minimizing per-op overhead (~150ns each).
    The last level emits fp16; one ScalarE copy upcasts min|max to fp32.
  - Mean via the Tensor engine: 64 normal fp16 matmuls against I/16
    accumulate transposed token chunks into one fp32 PSUM bank
    ([feat, span] layout), ScalarE copies PSUM->SBUF fp16, 4 matmuls
    against I transpose back, ScalarE copies to the fp32 result.
  - One [128, 2, 768] result tile ([min|max|mean] per span-tile) stored
    with one 768KB DMA per pair.
"""

import sys

import numpy as np

_TRN_REPO = "/opt/trn_rl_repo"

B, T, D, S = 16, 8192, 256, 512
L = T // S  # 16 tokens per span in the uniform layout
N_CORES = 8
BPC = B // N_CORES  # examples per core
P = 128  # SBUF partitions
TILES = S // P  # span-tiles per example
PAIRS = TILES // 2  # tile-pairs per example

_PROG_CACHE = {}


def _build_program():
    if _TRN_REPO not in sys.path:
        sys.path.insert(0, _TRN_REPO)
    from concourse import bacc, tile
    import concourse.mybir as mybir

    f32 = mybir.dt.float32
    f16 = mybir.dt.float16
    Alu = mybir.AluOpType

    nc = bacc.Bacc(
        "TRN2", target_bir_lowering=False, debug=False, enable_partition_id=False
    )
    x = nc.dram_tensor("x", [BPC, T, D], f16, kind="ExternalInput").ap()
    ident = nc.dram_tensor("ident", [P, 2 * P], f16, kind="ExternalInput").ap()
    # device output is fp16 (the host upcasts to fp32) — halves store traffic
    out = nc.dram_tensor("out", [BPC, S, 3 * D], f16, kind="ExternalOutput").ap()

    W = L * D  # free width per span-tile (4096)

    # [BPC, PAIRS, 128, 2, W] — partition p holds spans (2g+j)*128+p (j=0,1)
    xv = x.rearrange("b (g j p l) d -> b g p j (l d)", g=PAIRS, j=2, p=P, l=L)
    # output view matching the pair layout
    ov = out.rearrange("b (g j p) f -> b g p j f", g=PAIRS, j=2, p=P)

    with tile.TileContext(nc) as tc:
        with (
            tc.tile_pool(name="xin", bufs=4) as xin_pool,
            tc.tile_pool(name="identp", bufs=1) as ident_pool,
            tc.tile_pool(name="acc", bufs=2, space="PSUM") as acc_pool,
            tc.tile_pool(name="back", bufs=2, space="PSUM") as back_pool,
            tc.tile_pool(name="mid", bufs=2) as mid_pool,
            tc.tile_pool(name="scratch", bufs=2) as scratch,
            tc.tile_pool(name="res", bufs=2) as res_pool,
        ):
            # ident[:, 0:128] = I, ident[:, 128:256] = I/16 (both fp16)
            # loaded via the (otherwise idle) SWDGE queue so the HWDGE
            # queues are free for x chunks at startup
            idt2 = ident_pool.tile([P, 2 * P], f16)
            nc.gpsimd.dma_start(out=idt2, in_=ident)
            idt = idt2[:, 0:P]
            idtS = idt2[:, P : 2 * P]

            for b in range(BPC):
                for g in range(PAIRS):
                    first = b == 0 and g == 0
                    last = b == BPC - 1 and g == PAIRS - 1
                    t = xin_pool.tile([P, 2, W], f16, tag="xin")
                    pidx = b * PAIRS + g
                    if first:
                        # warmup: j0 chunks on the sync queue, j1 chunks on
                        # the scalar queue CONCURRENTLY — two DMA streams
                        # fill the SDMA engines and the DVE starts early
                        bounds0 = [0, 512, 1024, 2048, 3072, W]
                        for lo, hi in zip(bounds0[:-1], bounds0[1:]):
                            nc.sync.dma_start(
                                out=t[:, 0, lo:hi], in_=xv[b, g][:, 0, lo:hi])
                        for q in range(2):
                            CW = W // 2
                            nc.scalar.dma_start(
                                out=t[:, 1, q * CW : (q + 1) * CW],
                                in_=xv[b, g][:, 1, q * CW : (q + 1) * CW])
                    elif pidx == 2:
                        # alternate queues so consecutive loads overlap
                        nc.scalar.dma_start(out=t, in_=xv[b, g])
                    else:
                        nc.sync.dma_start(out=t, in_=xv[b, g])

                    res2 = res_pool.tile([P, 2, 3 * D], f16, tag="res")

                    # --- min/max trees (DVE) ---
                    s1n = scratch.tile([P, 2, W // 2], f16, tag="s1n")
                    s1x = scratch.tile([P, 2, W // 2], f16, tag="s1x")
                    HW_ = W // 2
                    if first:
                        # L1 gated per DMA chunk (pairs tokens within each
                        # chunk — valid since min/max are commutative)
                        # interleaved to match the two concurrent DMA streams
                        pieces = [(0, 0, 512), (0, 512, 1024),
                                  (1, 0, W // 2), (0, 1024, 2048),
                                  (1, W // 2, W), (0, 2048, 3072),
                                  (0, 3072, W)]
                        for j, lo, hi in pieces:
                            E = (hi - lo) // 2
                            for s1, op in ((s1n, Alu.min), (s1x, Alu.max)):
                                nc.vector.tensor_tensor(
                                    out=s1[:, j, lo // 2 : hi // 2],
                                    in0=t[:, j, lo : lo + E],
                                    in1=t[:, j, lo + E : hi], op=op)
                    elif last:
                        # per-sub-tile L1 so sub-results store early
                        for j in range(2):
                            for s1, op in ((s1n, Alu.min), (s1x, Alu.max)):
                                nc.vector.tensor_tensor(
                                    out=s1[:, j, :], in0=t[:, j, 0:HW_],
                                    in1=t[:, j, HW_:W], op=op)
                    else:
                        for s1, op in ((s1n, Alu.min), (s1x, Alu.max)):
                            nc.vector.tensor_tensor(
                                out=s1, in0=t[:, :, 0:HW_],
                                in1=t[:, :, HW_:W], op=op)

                    s2n = scratch.tile([P, 2, W // 4], f16, tag="s2n")
                    s2x = scratch.tile([P, 2, W // 4], f16, tag="s2x")
                    s3n = scratch.tile([P, 2, W // 8], f16, tag="s3n")
                    s3x = scratch.tile([P, 2, W // 8], f16, tag="s3x")

                    stats = ((s1n, s2n, s3n, Alu.min, 0),
                             (s1x, s2x, s3x, Alu.max, D))

                    def levels(js, chain):
                        """L2..L4 for one stat over j-slice js."""
                        sa, sb, sc, op, off = chain
                        nc.vector.tensor_tensor(
                            out=sb[:, js, :], in0=sa[:, js, 0 : W // 4],
                            in1=sa[:, js, W // 4 : W // 2], op=op)
                        nc.vector.tensor_tensor(
                            out=sc[:, js, :], in0=sb[:, js, 0 : W // 8],
                            in1=sb[:, js, W // 8 : W // 4], op=op)
                        nc.vector.tensor_tensor(
                            out=res2[:, js, off : off + D], in0=sc[:, js, 0:D],
                            in1=sc[:, js, D : 2 * D], op=op)

                    if last:
                        # per (j, stat) so each 64KB result streams out the
                        # moment its tree finishes (minimal tail)
                        for j in range(2):
                            js = slice(j, j + 1)
                            for chain in stats:
                                off = chain[-1]
                                levels(js, chain)
                                nc.sync.dma_start(
                                    out=ov[b, g][:, js, off : off + D],
                                    in_=res2[:, js, off : off + D])
                    else:
                        for chain in stats:
                            levels(slice(None), chain)

                    # --- mean via PE ---
                    acc = acc_pool.tile([P, 4 * P], f32, tag="acc")
                    for j in range(2):
                        for h in range(2):
                            gidx = j * 2 + h
                            for tok in range(L):
                                c = 2 * tok + h
                                nc.tensor.matmul(
                                    out=acc[:, gidx * P : (gidx + 1) * P],
                                    lhsT=t[:, j, c * P : (c + 1) * P],
                                    rhs=idtS,
                                    start=(tok == 0),
                                    stop=(tok == L - 1),
                                )
                    mid = mid_pool.tile([P, 4 * P], f16, tag="mid")
                    nc.scalar.copy(out=mid, in_=acc)
                    backp = back_pool.tile([P, 4 * P], f32, tag="back")
                    for gidx in range(4):
                        nc.tensor.matmul(
                            out=backp[:, gidx * P : (gidx + 1) * P],
                            lhsT=mid[:, gidx * P : (gidx + 1) * P],
                            rhs=idt,
                            start=True,
                            stop=True,
                        )
                    # backp columns are ordered (j, h, d) == res2[:, j, 2D:3D]
                    nc.scalar.copy(out=res2[:, :, 2 * D : 3 * D], in_=backp)

                    # --- store (mean separate: it's ready before the trees;
                    # steady stores via the idle SWDGE queue, keeping the
                    # HWDGE queues free for loads / the low-latency tail) ---
                    if last:
                        nc.scalar.dma_start(
                            out=ov[b, g][:, :, 2 * D : 3 * D],
                            in_=res2[:, :, 2 * D : 3 * D])
                    else:
                        nc.gpsimd.dma_start(
                            out=ov[b, g][:, :, 2 * D : 3 * D],
                            in_=res2[:, :, 2 * D : 3 * D])
                        nc.gpsimd.dma_start(
                            out=ov[b, g][:, :, 0 : 2 * D],
                            in_=res2[:, :, 0 : 2 * D])
    nc.compile()
    return nc


def _get_program():
    if "nc" not in _PROG_CACHE:
        _PROG_CACHE["nc"] = _build_program()
    return _PROG_CACHE["nc"]


def _ensure_ntff_hook():
    """Register the axon NTFF profiling hook if the image lacks
    antenv.axon_hooks (replicates trn_boot._ntff_profile_via_ctypes)."""
    try:
        from antenv.axon_hooks import get_axon_ntff_profile_hook  # noqa: F401

        return
    except ImportError:
        pass
    import contextlib
    import ctypes
    import types

    try:
        import antenv
    except ImportError:
        return

    so_path = "/opt/axon/libaxon_pjrt.so"
    mod = types.ModuleType("antenv.axon_hooks")
    holder = {"hook": None}
    mod.set_axon_ntff_profile_hook = lambda h: holder.__setitem__("hook", h)
    mod.get_axon_ntff_profile_hook = lambda: holder["hook"]
    sys.modules["antenv.axon_hooks"] = mod
    antenv.axon_hooks = mod

    try:
        lib = ctypes.CDLL(so_path)
    except OSError:
        return
    if not hasattr(lib, "axon_start_nrt_profile"):
        return
    lib.axon_start_nrt_profile.argtypes = [
        ctypes.POINTER(ctypes.c_int64),
        ctypes.c_size_t,
    ]
    lib.axon_start_nrt_profile.restype = ctypes.c_int64
    lib.axon_stop_nrt_profile.argtypes = [ctypes.c_char_p]
    lib.axon_stop_nrt_profile.restype = ctypes.c_int64

    @contextlib.contextmanager
    def _hook(output_dir, device_ids):
        import jax

        jax.devices()
        if device_ids:
            ids = (ctypes.c_int64 * len(device_ids))(*device_ids)
            rc = lib.axon_start_nrt_profile(ids, len(device_ids))
        else:
            rc = lib.axon_start_nrt_profile(None, 0)
        if rc != 0:
            raise RuntimeError(f"axon_start_nrt_profile rc={rc}")
        try:
            yield
        finally:
            n = lib.axon_stop_nrt_profile(str(output_dir).encode())
            if n < 0:
                raise RuntimeError(f"axon_stop_nrt_profile rc={n}")
            if n == 0:
                print(f"profile: 0 files written to {output_dir}", file=sys.stderr)

    mod.set_axon_ntff_profile_hook(_hook)


def _run_device(x, trace=False):
    """x: [B, T, D] float32 (uniform span layout). Returns ([B, S, 3D], exec_ns)."""
    if _TRN_REPO not in sys.path:
        sys.path.insert(0, _TRN_REPO)
    if trace:
        _ensure_ntff_hook()
    from concourse.bass_utils import run_bass_kernel_spmd

    nc = _get_program()
    x16 = x.astype(np.float16)
    eye = np.eye(P, dtype=np.float16)
    ident = np.concatenate([eye, eye / np.float16(L)], axis=1)
    in_maps = [
        {"x": np.ascontiguousarray(x16[c * BPC : (c + 1) * BPC]), "ident": ident}
        for c in range(N_CORES)
    ]
    res = run_bass_kernel_spmd(
        nc, in_maps, core_ids=list(range(N_CORES)), trace=trace
    )
    out = np.concatenate(
        [res.results[c]["out"] for c in range(N_CORES)], axis=0
    ).astype(np.float32)
    # Output order per row is [min | max | mean]; reference order is
    # [smin, smax, mean] — identical.
    return out, res.exec_time_ns


def _is_uniform(span_idxs):
    if span_idxs.shape != (B, S, 2):
        return False
    starts = np.arange(S, dtype=np.int64) * L
    return bool(
        np.all(span_idxs[..., 0] == starts[None, :])
        and np.all(span_idxs[..., 1] == starts[None, :] + L)
    )


def _fallback(x, lengths, span_idxs):
    """Exact numpy port of the reference semantics (general spans)."""
    Bn, Tn, Dn = x.shape
    Sn = span_idxs.shape[1]
    starts = span_idxs[..., 0]
    ends = span_idxs[..., 1]
    t = np.arange(Tn)
    out = np.zeros((Bn, Sn, 3 * Dn), np.float32)
    for b in range(Bn):
        seg = np.searchsorted(starts[b], t, side="right") - 1
        seg_c = np.clip(seg, 0, Sn - 1)
        in_span = (seg >= 0) & (t < ends[b][seg_c])
        valid_row = np.arange(Sn) < lengths[b]
        tok_valid = in_span & valid_row[seg_c]
        sid = np.where(tok_valid, seg_c, Sn)
        order = np.argsort(sid, kind="stable")
        ssorted = sid[order]
        xs = x[b][order]
        bounds = np.searchsorted(ssorted, np.arange(Sn + 1))
        for s in range(Sn):
            lo, hi = bounds[s], bounds[s + 1]
            if hi > lo:
                seg_x = xs[lo:hi]
                out[b, s, :Dn] = seg_x.min(axis=0)
                out[b, s, Dn : 2 * Dn] = seg_x.max(axis=0)
                out[b, s, 2 * Dn :] = seg_x.sum(axis=0, dtype=np.float32) / float(
                    hi - lo
                )
    return out


def kernel(x, lengths, span_idxs, _trace=False):
    x = np.asarray(x, dtype=np.float32)
    lengths = np.asarray(lengths, dtype=np.int32)
    span_idxs = np.asarray(span_idxs, dtype=np.int32)

    if x.shape == (B, T, D) and _is_uniform(span_idxs):
        out, exec_ns = _run_device(x, trace=_trace)
        row_ok = np.arange(S)[None, :] < lengths[:, None]
        if not row_ok.all():
            out = np.where(row_ok[..., None], out, np.float32(0.0))
        if _trace:
            return out, exec_ns
        return out

    out = _fallback(x, lengths, span_idxs)
    if _trace:
        return out, None
    return out


if __name__ == "__main__":
    rng = np.random.default_rng(0)
    x = rng.standard_normal((B, T, D), dtype=np.float32)
    starts = (np.arange(S, dtype=np.int32) * L)[None, :].repeat(B, 0)
    span_idxs = np.stack([starts, starts + L], axis=-1).astype(np.int32)
    lengths = np.full((B,), S, dtype=np.int32)
    got = kernel(x, lengths, span_idxs)
    xb = x.reshape(B, S, L, D)
    exp = np.concatenate(
        [xb.min(2), xb.max(2), xb.mean(2, dtype=np.float32)], axis=-1
    )
    err = np.abs(got - exp).max()
    print("self-test max abs err:", err, " rel:", err / np.abs(exp).max())


# revision 25
# speedup vs baseline: 1.0539x; 1.0539x over previous
"""Trainium2 Bass kernel for nn_Agg_57380763075323 (segment_reduce).

Computes, for each (batch, span): [min, max, mean] over the span's tokens of
x[B=16, T=8192, D=256], output [B, S=512, 3*D=768] float32.

Device fast path assumes the uniform span structure produced by
setup_inputs(): span s covers tokens [s*16, (s+1)*16) for all examples.
Anything else falls back to an exact numpy implementation of the reference
semantics (searchsorted-based segment assignment).

Sharding: data-parallel over batch; each of the 8 NeuronCores processes 2
examples. No cross-core communication.

Precision strategy: the output tolerance is rel_err < 2e-2; x is converted
to fp16 on the host during the shard step (rel err <= 2^-11 ~ 5e-4).  This
halves HBM load traffic (the memory-bound roofline) and doubles DVE
tensor_tensor throughput (2x_1P perf mode for 16-bit dtypes).

Device algorithm per core (2 examples, each [8192, 256] fp16), working in
PAIRS of 128-span tiles ([128, 2, 4096] fp16 = one 2MB DMA):
  - min and max via pairwise log-trees of fp16 tensor_tensor ops on the
    Vector engine; each tree level is ONE batched op across the pair
    (3D access patterns), minimizing per-op overhead (~150ns each).
    The last level emits fp16; one ScalarE copy upcasts min|max to fp32.
  - Mean via the Tensor engine: 64 normal fp16 matmuls against I/16
    accumulate transposed token chunks into one fp32 PSUM bank
    ([feat, span] layout), ScalarE copies PSUM->SBUF fp16, 4 matmuls
    against I transpose back, ScalarE copies to the fp32 result.
  - One [128, 2, 768] result tile ([min|max|mean] per span-tile) stored
    with one 768KB DMA per pair.
"""

import sys

import numpy as np

_TRN_REPO = "/opt/trn_rl_repo"

B, T, D, S = 16, 8192, 256, 512
L = T // S  # 16 tokens per span in the uniform layout
N_CORES = 8
BPC = B // N_CORES  # examples per core
P = 128  # SBUF partitions
TILES = S // P  # span-tiles per example
PAIRS = TILES // 2  # tile-pairs per example

_PROG_CACHE = {}


def _build_program():
    if _TRN_REPO not in sys.path:
        sys.path.insert(0, _TRN_REPO)
    from concourse import bacc, tile
    import concourse.mybir as mybir

    f32 = mybir.dt.float32
    f16 = mybir.dt.float16
    Alu = mybir.AluOpType

    nc = bacc.Bacc(
        "TRN2", target_bir_lowering=False, debug=False, enable_partition_id=False
    )
    x = nc.dram_tensor("x", [BPC, T, D], f16, kind="ExternalInput").ap()
    ident = nc.dram_tensor("ident", [P, 2 * P], f16, kind="ExternalInput").ap()
    # device output is fp16 (the host upcasts to fp32) — halves store traffic
    out = nc.dram_tensor("out", [BPC, S, 3 * D], f16, kind="ExternalOutput").ap()

    W = L * D  # free width per span-tile (4096)

    # [BPC, PAIRS, 128, 2, W] — partition p holds spans (2g+j)*128+p (j=0,1)
    xv = x.rearrange("b (g j p l) d -> b g p j (l d)", g=PAIRS, j=2, p=P, l=L)
    # output view matching the pair layout
    ov = out.rearrange("b (g j p) f -> b g p j f", g=PAIRS, j=2, p=P)

    with tile.TileContext(nc) as tc:
        with (
            tc.tile_pool(name="xin", bufs=4) as xin_pool,
            tc.tile_pool(name="identp", bufs=1) as ident_pool,
            tc.tile_pool(name="acc", bufs=2, space="PSUM") as acc_pool,
            tc.tile_pool(name="back", bufs=2, space="PSUM") as back_pool,
            tc.tile_pool(name="mid", bufs=2) as mid_pool,
            tc.tile_pool(name="scratch", bufs=2) as scratch,
            tc.tile_pool(name="res", bufs=2) as res_pool,
        ):
            # ident[:, 0:128] = I, ident[:, 128:256] = I/16 (both fp16)
            # loaded via the (otherwise idle) SWDGE queue so the HWDGE
            # queues are free for x chunks at startup
            idt2 = ident_pool.tile([P, 2 * P], f16)
            nc.gpsimd.dma_start(out=idt2, in_=ident)
            idt = idt2[:, 0:P]
            idtS = idt2[:, P : 2 * P]

            # ---- pass 1: issue ALL load triggers up front (the 4 pair
            # tiles coexist: bufs=4).  The x stream stays on the sync
            # queue in consumption order (FIFO per ring = full ring
            # bandwidth per transfer); only pair0's j1 half rides the
            # scalar queue so the warmup fills from two streams. ----
            tiles = []
            for b in range(BPC):
                for g in range(PAIRS):
                    pidx = b * PAIRS + g
                    t = xin_pool.tile([P, 2, W], f16, tag="xin")
                    tiles.append(t)
                    if pidx == 0:
                        bounds0 = [0, 1024, 2048, 3072, W]
                        for lo, hi in zip(bounds0[:-1], bounds0[1:]):
                            nc.sync.dma_start(
                                out=t[:, 0, lo:hi], in_=xv[b, g][:, 0, lo:hi])
                        for q in range(2):
                            CW = W // 2
                            nc.scalar.dma_start(
                                out=t[:, 1, q * CW : (q + 1) * CW],
                                in_=xv[b, g][:, 1, q * CW : (q + 1) * CW])
                    elif pidx in (1, 2):
                        # per-sub-tile loads: the DVE's L1 gates on half
                        # the pair while the DMA pipeline is still ramping
                        nc.sync.dma_start(out=t[:, 0], in_=xv[b, g][:, 0])
                        nc.sync.dma_start(out=t[:, 1], in_=xv[b, g][:, 1])
                    else:
                        nc.sync.dma_start(out=t, in_=xv[b, g])

            # ---- pass 2: compute + stores ----
            for b in range(BPC):
                for g in range(PAIRS):
                    first = b == 0 and g == 0
                    last = b == BPC - 1 and g == PAIRS - 1
                    pidx = b * PAIRS + g
                    t = tiles[pidx]

                    res2 = res_pool.tile([P, 2, 3 * D], f16, tag="res")

                    # --- min/max trees (DVE) ---
                    s1n = scratch.tile([P, 2, W // 2], f16, tag="s1n")
                    s1x = scratch.tile([P, 2, W // 2], f16, tag="s1x")
                    HW_ = W // 2
                    if first:
                        # L1 gated per DMA chunk (pairs tokens within each
                        # chunk — valid since min/max are commutative)
                        # interleaved to match the two concurrent DMA streams
                        pieces = [(0, 0, 1024), (1, 0, W // 2),
                                  (0, 1024, 2048), (0, 2048, 3072),
                                  (1, W // 2, W), (0, 3072, W)]
                        for j, lo, hi in pieces:
                            E = (hi - lo) // 2
                            for s1, op in ((s1n, Alu.min), (s1x, Alu.max)):
                                nc.vector.tensor_tensor(
                                    out=s1[:, j, lo // 2 : hi // 2],
                                    in0=t[:, j, lo : lo + E],
                                    in1=t[:, j, lo + E : hi], op=op)
                    elif last or pidx in (1, 2):
                        # per-sub-tile L1 (finer DMA gating / early stores)
                        for j in range(2):
                            for s1, op in ((s1n, Alu.min), (s1x, Alu.max)):
                                nc.vector.tensor_tensor(
                                    out=s1[:, j, :], in0=t[:, j, 0:HW_],
                                    in1=t[:, j, HW_:W], op=op)
                    else:
                        for s1, op in ((s1n, Alu.min), (s1x, Alu.max)):
                            nc.vector.tensor_tensor(
                                out=s1, in0=t[:, :, 0:HW_],
                                in1=t[:, :, HW_:W], op=op)

                    s2n = scratch.tile([P, 2, W // 4], f16, tag="s2n")
                    s2x = scratch.tile([P, 2, W // 4], f16, tag="s2x")
                    s3n = scratch.tile([P, 2, W // 8], f16, tag="s3n")
                    s3x = scratch.tile([P, 2, W // 8], f16, tag="s3x")

                    stats = ((s1n, s2n, s3n, Alu.min, 0),
                             (s1x, s2x, s3x, Alu.max, D))

                    def levels(js, chain):
                        """L2..L4 for one stat over j-slice js."""
                        sa, sb, sc, op, off = chain
                        nc.vector.tensor_tensor(
                            out=sb[:, js, :], in0=sa[:, js, 0 : W // 4],
                            in1=sa[:, js, W // 4 : W // 2], op=op)
                        nc.vector.tensor_tensor(
                            out=sc[:, js, :], in0=sb[:, js, 0 : W // 8],
                            in1=sb[:, js, W // 8 : W // 4], op=op)
                        nc.vector.tensor_tensor(
                            out=res2[:, js, off : off + D], in0=sc[:, js, 0:D],
                            in1=sc[:, js, D : 2 * D], op=op)

                    if last:
                        # per (j, stat) so each 64KB result streams out the
                        # moment its tree finishes (minimal tail)
                        for j in range(2):
                            js = slice(j, j + 1)
                            for chain in stats:
                                off = chain[-1]
                                levels(js, chain)
                                nc.sync.dma_start(
                                    out=ov[b, g][:, js, off : off + D],
                                    in_=res2[:, js, off : off + D])
                    else:
                        for chain in stats:
                            levels(slice(None), chain)

                    # --- mean via PE ---
                    acc = acc_pool.tile([P, 4 * P], f32, tag="acc")
                    for j in range(2):
                        for h in range(2):
                            gidx = j * 2 + h
                            for tok in range(L):
                                c = 2 * tok + h
                                nc.tensor.matmul(
                                    out=acc[:, gidx * P : (gidx + 1) * P],
                                    lhsT=t[:, j, c * P : (c + 1) * P],
                                    rhs=idtS,
                                    start=(tok == 0),
                                    stop=(tok == L - 1),
                                )
                    mid = mid_pool.tile([P, 4 * P], f16, tag="mid")
                    nc.scalar.copy(out=mid, in_=acc)
                    backp = back_pool.tile([P, 4 * P], f32, tag="back")
                    for gidx in range(4):
                        nc.tensor.matmul(
                            out=backp[:, gidx * P : (gidx + 1) * P],
                            lhsT=mid[:, gidx * P : (gidx + 1) * P],
                            rhs=idt,
                            start=True,
                            stop=True,
                        )
                    # backp columns are ordered (j, h, d) == res2[:, j, 2D:3D]
                    nc.scalar.copy(out=res2[:, :, 2 * D : 3 * D], in_=backp)

                    # --- store (mean on scalar — it's ready early; min/max
                    # on sync, whose queue has drained its load triggers) ---
                    nc.scalar.dma_start(
                        out=ov[b, g][:, :, 2 * D : 3 * D],
                        in_=res2[:, :, 2 * D : 3 * D])
                    if not last:
                        nc.sync.dma_start(
                            out=ov[b, g][:, :, 0 : 2 * D],
                            in_=res2[:, :, 0 : 2 * D])
    nc.compile()
    return nc


def _get_program():
    if "nc" not in _PROG_CACHE:
        _PROG_CACHE["nc"] = _build_program()
    return _PROG_CACHE["nc"]


def _ensure_ntff_hook():
    """Register the axon NTFF profiling hook if the image lacks
    antenv.axon_hooks (replicates trn_boot._ntff_profile_via_ctypes)."""
    try:
        from antenv.axon_hooks import get_axon_ntff_profile_hook  # noqa: F401

        return
    except ImportError:
        pass
    import contextlib
    import ctypes
    import types

    try:
        import antenv
    except ImportError:
        return

    so_path = "/opt/axon/libaxon_pjrt.so"
    mod = types.ModuleType("antenv.axon_hooks")
    holder = {"hook": None}
    mod.set_axon_ntff_profile_hook = lambda h: holder.__setitem__("hook", h)
    mod.get_axon_ntff_profile_hook = lambda: holder["hook"]
    sys.modules["antenv.axon_hooks"] = mod
    antenv.axon_hooks = mod

    try:
        lib = ctypes.CDLL(so_path)
    except OSError:
        return
    if not hasattr(lib, "axon_start_nrt_profile"):
        return
    lib.axon_start_nrt_profile.argtypes = [
        ctypes.POINTER(ctypes.c_int64),
        ctypes.c_size_t,
    ]
    lib.axon_start_nrt_profile.restype = ctypes.c_int64
    lib.axon_stop_nrt_profile.argtypes = [ctypes.c_char_p]
    lib.axon_stop_nrt_profile.restype = ctypes.c_int64

    @contextlib.contextmanager
    def _hook(output_dir, device_ids):
        import jax

        jax.devices()
        if device_ids:
            ids = (ctypes.c_int64 * len(device_ids))(*device_ids)
            rc = lib.axon_start_nrt_profile(ids, len(device_ids))
        else:
            rc = lib.axon_start_nrt_profile(None, 0)
        if rc != 0:
            raise RuntimeError(f"axon_start_nrt_profile rc={rc}")
        try:
            yield
        finally:
            n = lib.axon_stop_nrt_profile(str(output_dir).encode())
            if n < 0:
                raise RuntimeError(f"axon_stop_nrt_profile rc={n}")
            if n == 0:
                print(f"profile: 0 files written to {output_dir}", file=sys.stderr)

    mod.set_axon_ntff_profile_hook(_hook)


def _run_device(x, trace=False):
    """x: [B, T, D] float32 (uniform span layout). Returns ([B, S, 3D], exec_ns)."""
    if _TRN_REPO not in sys.path:
        sys.path.insert(0, _TRN_REPO)
    if trace:
        _ensure_ntff_hook()
    from concourse.bass_utils import run_bass_kernel_spmd

    nc = _get_program()
    x16 = x.astype(np.float16)
    eye = np.eye(P, dtype=np.float16)
    ident = np.concatenate([eye, eye / np.float16(L)], axis=1)
    in_maps = [
        {"x": np.ascontiguousarray(x16[c * BPC : (c + 1) * BPC]), "ident": ident}
        for c in range(N_CORES)
    ]
    res = run_bass_kernel_spmd(
        nc, in_maps, core_ids=list(range(N_CORES)), trace=trace
    )
    out = np.concatenate(
        [res.results[c]["out"] for c in range(N_CORES)], axis=0
    ).astype(np.float32)
    # Output order per row is [min | max | mean]; reference order is
    # [smin, smax, mean] — identical.
    return out, res.exec_time_ns


def _is_uniform(span_idxs):
    if span_idxs.shape != (B, S, 2):
        return False
    starts = np.arange(S, dtype=np.int64) * L
    return bool(
        np.all(span_idxs[..., 0] == starts[None, :])
        and np.all(span_idxs[..., 1] == starts[None, :] + L)
    )


def _fallback(x, lengths, span_idxs):
    """Exact numpy port of the reference semantics (general spans)."""
    Bn, Tn, Dn = x.shape
    Sn = span_idxs.shape[1]
    starts = span_idxs[..., 0]
    ends = span_idxs[..., 1]
    t = np.arange(Tn)
    out = np.zeros((Bn, Sn, 3 * Dn), np.float32)
    for b in range(Bn):
        seg = np.searchsorted(starts[b], t, side="right") - 1
        seg_c = np.clip(seg, 0, Sn - 1)
        in_span = (seg >= 0) & (t < ends[b][seg_c])
        valid_row = np.arange(Sn) < lengths[b]
        tok_valid = in_span & valid_row[seg_c]
        sid = np.where(tok_valid, seg_c, Sn)
        order = np.argsort(sid, kind="stable")
        ssorted = sid[order]
        xs = x[b][order]
        bounds = np.searchsorted(ssorted, np.arange(Sn + 1))
        for s in range(Sn):
            lo, hi = bounds[s], bounds[s + 1]
            if hi > lo:
                seg_x = xs[lo:hi]
                out[b, s, :Dn] = seg_x.min(axis=0)
                out[b, s, Dn : 2 * Dn] = seg_x.max(axis=0)
                out[b, s, 2 * Dn :] = seg_x.sum(axis=0, dtype=np.float32) / float(
                    hi - lo
                )
    return out


def kernel(x, lengths, span_idxs, _trace=False):
    x = np.asarray(x, dtype=np.float32)
    lengths = np.asarray(lengths, dtype=np.int32)
    span_idxs = np.asarray(span_idxs, dtype=np.int32)

    if x.shape == (B, T, D) and _is_uniform(span_idxs):
        out, exec_ns = _run_device(x, trace=_trace)
        row_ok = np.arange(S)[None, :] < lengths[:, None]
        if not row_ok.all():
            out = np.where(row_ok[..., None], out, np.float32(0.0))
        if _trace:
            return out, exec_ns
        return out

    out = _fallback(x, lengths, span_idxs)
    if _trace:
        return out, None
    return out


if __name__ == "__main__":
    rng = np.random.default_rng(0)
    x = rng.standard_normal((B, T, D), dtype=np.float32)
    starts = (np.arange(S, dtype=np.int32) * L)[None, :].repeat(B, 0)
    span_idxs = np.stack([starts, starts + L], axis=-1).astype(np.int32)
    lengths = np.full((B,), S, dtype=np.int32)
    got = kernel(x, lengths, span_idxs)
    xb = x.reshape(B, S, L, D)
    exp = np.concatenate(
        [xb.min(2), xb.max(2), xb.mean(2, dtype=np.float32)], axis=-1
    )
    err = np.abs(got - exp).max()
    print("self-test max abs err:", err, " rel:", err / np.abs(exp).max())


# revision 27
# speedup vs baseline: 1.0592x; 1.0050x over previous
"""Trainium2 Bass kernel for nn_Agg_57380763075323 (segment_reduce).

Computes, for each (batch, span): [min, max, mean] over the span's tokens of
x[B=16, T=8192, D=256], output [B, S=512, 3*D=768] float32.

Device fast path assumes the uniform span structure produced by
setup_inputs(): span s covers tokens [s*16, (s+1)*16) for all examples.
Anything else falls back to an exact numpy implementation of the reference
semantics (searchsorted-based segment assignment).

Sharding: data-parallel over batch; each of the 8 NeuronCores processes 2
examples. No cross-core communication.

Precision strategy: the output tolerance is rel_err < 2e-2; x is converted
to fp16 on the host during the shard step (rel err <= 2^-11 ~ 5e-4).  This
halves HBM load traffic (the memory-bound roofline) and doubles DVE
tensor_tensor throughput (2x_1P perf mode for 16-bit dtypes).

Device algorithm per core (2 examples, each [8192, 256] fp16), working in
PAIRS of 128-span tiles ([128, 2, 4096] fp16 = one 2MB DMA):
  - min and max via pairwise log-trees of fp16 tensor_tensor ops on the
    Vector engine; each tree level is ONE batched op across the pair
    (3D access patterns), minimizing per-op overhead (~150ns each).
    The last level emits fp16; one ScalarE copy upcasts min|max to fp32.
  - Mean via the Tensor engine: 64 normal fp16 matmuls against I/16
    accumulate transposed token chunks into one fp32 PSUM bank
    ([feat, span] layout), ScalarE copies PSUM->SBUF fp16, 4 matmuls
    against I transpose back, ScalarE copies to the fp32 result.
  - One [128, 2, 768] result tile ([min|max|mean] per span-tile) stored
    with one 768KB DMA per pair.
"""

import sys

import numpy as np

_TRN_REPO = "/opt/trn_rl_repo"

B, T, D, S = 16, 8192, 256, 512
L = T // S  # 16 tokens per span in the uniform layout
N_CORES = 8
BPC = B // N_CORES  # examples per core
P = 128  # SBUF partitions
TILES = S // P  # span-tiles per example
PAIRS = TILES // 2  # tile-pairs per example

_PROG_CACHE = {}


def _build_program():
    if _TRN_REPO not in sys.path:
        sys.path.insert(0, _TRN_REPO)
    from concourse import bacc, tile
    import concourse.mybir as mybir

    f32 = mybir.dt.float32
    f16 = mybir.dt.float16
    Alu = mybir.AluOpType

    nc = bacc.Bacc(
        "TRN2", target_bir_lowering=False, debug=False, enable_partition_id=False
    )
    x = nc.dram_tensor("x", [BPC, T, D], f16, kind="ExternalInput").ap()
    ident = nc.dram_tensor("ident", [P, 2 * P], f16, kind="ExternalInput").ap()
    # device output is fp16 (the host upcasts to fp32) — halves store traffic
    out = nc.dram_tensor("out", [BPC, S, 3 * D], f16, kind="ExternalOutput").ap()

    W = L * D  # free width per span-tile (4096)

    # [BPC, PAIRS, 128, 2, W] — partition p holds spans (2g+j)*128+p (j=0,1)
    xv = x.rearrange("b (g j p l) d -> b g p j (l d)", g=PAIRS, j=2, p=P, l=L)
    # output view matching the pair layout
    ov = out.rearrange("b (g j p) f -> b g p j f", g=PAIRS, j=2, p=P)

    with tile.TileContext(nc) as tc:
        with (
            tc.tile_pool(name="xin", bufs=4) as xin_pool,
            tc.tile_pool(name="identp", bufs=1) as ident_pool,
            tc.tile_pool(name="acc", bufs=2, space="PSUM") as acc_pool,
            tc.tile_pool(name="back", bufs=2, space="PSUM") as back_pool,
            tc.tile_pool(name="mid", bufs=2) as mid_pool,
            tc.tile_pool(name="scratch", bufs=2) as scratch,
            tc.tile_pool(name="res", bufs=2) as res_pool,
        ):
            # ident[:, 0:128] = I, ident[:, 128:256] = I/16 (both fp16)
            # loaded via the (otherwise idle) SWDGE queue so the HWDGE
            # queues are free for x chunks at startup
            idt2 = ident_pool.tile([P, 2 * P], f16)
            nc.gpsimd.dma_start(out=idt2, in_=ident)
            idt = idt2[:, 0:P]
            idtS = idt2[:, P : 2 * P]

            # ---- pass 1: issue ALL load triggers up front (the 4 pair
            # tiles coexist: bufs=4).  The x stream stays on the sync
            # queue in consumption order (FIFO per ring = full ring
            # bandwidth per transfer); only pair0's j1 half rides the
            # scalar queue so the warmup fills from two streams. ----
            tiles = []
            for b in range(BPC):
                for g in range(PAIRS):
                    pidx = b * PAIRS + g
                    t = xin_pool.tile([P, 2, W], f16, tag="xin")
                    tiles.append(t)
                    if pidx == 0:
                        # warmup chunks, all sequential on the sync ring so
                        # nothing competes with the critical first chunk
                        # (chunks stay >=1KB per partition row — smaller
                        # rows corrupt via the SDMA read-modify-write path)
                        bounds0 = [0, 512, 1024, 2048, 3072, W]
                        for lo, hi in zip(bounds0[:-1], bounds0[1:]):
                            nc.sync.dma_start(
                                out=t[:, 0, lo:hi], in_=xv[b, g][:, 0, lo:hi])
                        for q in range(2):
                            CW = W // 2
                            nc.sync.dma_start(
                                out=t[:, 1, q * CW : (q + 1) * CW],
                                in_=xv[b, g][:, 1, q * CW : (q + 1) * CW])
                    elif pidx in (1, 2):
                        # per-sub-tile loads: the DVE's L1 gates on half
                        # the pair while the DMA pipeline is still ramping
                        nc.sync.dma_start(out=t[:, 0], in_=xv[b, g][:, 0])
                        nc.sync.dma_start(out=t[:, 1], in_=xv[b, g][:, 1])
                    else:
                        nc.sync.dma_start(out=t, in_=xv[b, g])

            # ---- pass 2: compute + stores ----
            for b in range(BPC):
                for g in range(PAIRS):
                    first = b == 0 and g == 0
                    last = b == BPC - 1 and g == PAIRS - 1
                    pidx = b * PAIRS + g
                    t = tiles[pidx]

                    res2 = res_pool.tile([P, 2, 3 * D], f16, tag="res")

                    # --- min/max trees (DVE) ---
                    s1n = scratch.tile([P, 2, W // 2], f16, tag="s1n")
                    s1x = scratch.tile([P, 2, W // 2], f16, tag="s1x")
                    HW_ = W // 2
                    if first:
                        # L1 gated per DMA chunk (pairs tokens within each
                        # chunk — valid since min/max are commutative)
                        pieces = [(0, 0, 512), (0, 512, 1024),
                                  (0, 1024, 2048), (0, 2048, 3072),
                                  (0, 3072, W), (1, 0, W // 2),
                                  (1, W // 2, W)]
                        for j, lo, hi in pieces:
                            E = (hi - lo) // 2
                            for s1, op in ((s1n, Alu.min), (s1x, Alu.max)):
                                nc.vector.tensor_tensor(
                                    out=s1[:, j, lo // 2 : hi // 2],
                                    in0=t[:, j, lo : lo + E],
                                    in1=t[:, j, lo + E : hi], op=op)
                    elif last or pidx in (1, 2):
                        # per-sub-tile L1 (finer DMA gating / early stores)
                        for j in range(2):
                            for s1, op in ((s1n, Alu.min), (s1x, Alu.max)):
                                nc.vector.tensor_tensor(
                                    out=s1[:, j, :], in0=t[:, j, 0:HW_],
                                    in1=t[:, j, HW_:W], op=op)
                    else:
                        for s1, op in ((s1n, Alu.min), (s1x, Alu.max)):
                            nc.vector.tensor_tensor(
                                out=s1, in0=t[:, :, 0:HW_],
                                in1=t[:, :, HW_:W], op=op)

                    s2n = scratch.tile([P, 2, W // 4], f16, tag="s2n")
                    s2x = scratch.tile([P, 2, W // 4], f16, tag="s2x")
                    s3n = scratch.tile([P, 2, W // 8], f16, tag="s3n")
                    s3x = scratch.tile([P, 2, W // 8], f16, tag="s3x")

                    stats = ((s1n, s2n, s3n, Alu.min, 0),
                             (s1x, s2x, s3x, Alu.max, D))

                    def levels(js, chain):
                        """L2..L4 for one stat over j-slice js."""
                        sa, sb, sc, op, off = chain
                        nc.vector.tensor_tensor(
                            out=sb[:, js, :], in0=sa[:, js, 0 : W // 4],
                            in1=sa[:, js, W // 4 : W // 2], op=op)
                        nc.vector.tensor_tensor(
                            out=sc[:, js, :], in0=sb[:, js, 0 : W // 8],
                            in1=sb[:, js, W // 8 : W // 4], op=op)
                        nc.vector.tensor_tensor(
                            out=res2[:, js, off : off + D], in0=sc[:, js, 0:D],
                            in1=sc[:, js, D : 2 * D], op=op)

                    if last:
                        # per (j, stat) so each 64KB result streams out the
                        # moment its tree finishes (minimal tail)
                        for j in range(2):
                            js = slice(j, j + 1)
                            for chain in stats:
                                off = chain[-1]
                                levels(js, chain)
                                nc.sync.dma_start(
                                    out=ov[b, g][:, js, off : off + D],
                                    in_=res2[:, js, off : off + D])
                    else:
                        for chain in stats:
                            levels(slice(None), chain)

                    # --- mean via PE ---
                    acc = acc_pool.tile([P, 4 * P], f32, tag="acc")
                    for j in range(2):
                        for h in range(2):
                            gidx = j * 2 + h
                            for tok in range(L):
                                c = 2 * tok + h
                                nc.tensor.matmul(
                                    out=acc[:, gidx * P : (gidx + 1) * P],
                                    lhsT=t[:, j, c * P : (c + 1) * P],
                                    rhs=idtS,
                                    start=(tok == 0),
                                    stop=(tok == L - 1),
                                )
                    mid = mid_pool.tile([P, 4 * P], f16, tag="mid")
                    nc.scalar.copy(out=mid, in_=acc)
                    backp = back_pool.tile([P, 4 * P], f32, tag="back")
                    for gidx in range(4):
                        nc.tensor.matmul(
                            out=backp[:, gidx * P : (gidx + 1) * P],
                            lhsT=mid[:, gidx * P : (gidx + 1) * P],
                            rhs=idt,
                            start=True,
                            stop=True,
                        )
                    # backp columns are ordered (j, h, d) == res2[:, j, 2D:3D]
                    nc.scalar.copy(out=res2[:, :, 2 * D : 3 * D], in_=backp)

                    # --- store (mean on scalar — it's ready early; min/max
                    # on sync, whose queue has drained its load triggers) ---
                    nc.scalar.dma_start(
                        out=ov[b, g][:, :, 2 * D : 3 * D],
                        in_=res2[:, :, 2 * D : 3 * D])
                    if not last:
                        nc.sync.dma_start(
                            out=ov[b, g][:, :, 0 : 2 * D],
                            in_=res2[:, :, 0 : 2 * D])
    nc.compile()
    return nc


def _get_program():
    if "nc" not in _PROG_CACHE:
        _PROG_CACHE["nc"] = _build_program()
    return _PROG_CACHE["nc"]


def _ensure_ntff_hook():
    """Register the axon NTFF profiling hook if the image lacks
    antenv.axon_hooks (replicates trn_boot._ntff_profile_via_ctypes)."""
    try:
        from antenv.axon_hooks import get_axon_ntff_profile_hook  # noqa: F401

        return
    except ImportError:
        pass
    import contextlib
    import ctypes
    import types

    try:
        import antenv
    except ImportError:
        return

    so_path = "/opt/axon/libaxon_pjrt.so"
    mod = types.ModuleType("antenv.axon_hooks")
    holder = {"hook": None}
    mod.set_axon_ntff_profile_hook = lambda h: holder.__setitem__("hook", h)
    mod.get_axon_ntff_profile_hook = lambda: holder["hook"]
    sys.modules["antenv.axon_hooks"] = mod
    antenv.axon_hooks = mod

    try:
        lib = ctypes.CDLL(so_path)
    except OSError:
        return
    if not hasattr(lib, "axon_start_nrt_profile"):
        return
    lib.axon_start_nrt_profile.argtypes = [
        ctypes.POINTER(ctypes.c_int64),
        ctypes.c_size_t,
    ]
    lib.axon_start_nrt_profile.restype = ctypes.c_int64
    lib.axon_stop_nrt_profile.argtypes = [ctypes.c_char_p]
    lib.axon_stop_nrt_profile.restype = ctypes.c_int64

    @contextlib.contextmanager
    def _hook(output_dir, device_ids):
        import jax

        jax.devices()
        if device_ids:
            ids = (ctypes.c_int64 * len(device_ids))(*device_ids)
            rc = lib.axon_start_nrt_profile(ids, len(device_ids))
        else:
            rc = lib.axon_start_nrt_profile(None, 0)
        if rc != 0:
            raise RuntimeError(f"axon_start_nrt_profile rc={rc}")
        try:
            yield
        finally:
            n = lib.axon_stop_nrt_profile(str(output_dir).encode())
            if n < 0:
                raise RuntimeError(f"axon_stop_nrt_profile rc={n}")
            if n == 0:
                print(f"profile: 0 files written to {output_dir}", file=sys.stderr)

    mod.set_axon_ntff_profile_hook(_hook)


def _run_device(x, trace=False):
    """x: [B, T, D] float32 (uniform span layout). Returns ([B, S, 3D], exec_ns)."""
    if _TRN_REPO not in sys.path:
        sys.path.insert(0, _TRN_REPO)
    if trace:
        _ensure_ntff_hook()
    from concourse.bass_utils import run_bass_kernel_spmd

    nc = _get_program()
    x16 = x.astype(np.float16)
    eye = np.eye(P, dtype=np.float16)
    ident = np.concatenate([eye, eye / np.float16(L)], axis=1)
    in_maps = [
        {"x": np.ascontiguousarray(x16[c * BPC : (c + 1) * BPC]), "ident": ident}
        for c in range(N_CORES)
    ]
    res = run_bass_kernel_spmd(
        nc, in_maps, core_ids=list(range(N_CORES)), trace=trace
    )
    out = np.concatenate(
        [res.results[c]["out"] for c in range(N_CORES)], axis=0
    ).astype(np.float32)
    # Output order per row is [min | max | mean]; reference order is
    # [smin, smax, mean] — identical.
    return out, res.exec_time_ns


def _is_uniform(span_idxs):
    if span_idxs.shape != (B, S, 2):
        return False
    starts = np.arange(S, dtype=np.int64) * L
    return bool(
        np.all(span_idxs[..., 0] == starts[None, :])
        and np.all(span_idxs[..., 1] == starts[None, :] + L)
    )


def _fallback(x, lengths, span_idxs):
    """Exact numpy port of the reference semantics (general spans)."""
    Bn, Tn, Dn = x.shape
    Sn = span_idxs.shape[1]
    starts = span_idxs[..., 0]
    ends = span_idxs[..., 1]
    t = np.arange(Tn)
    out = np.zeros((Bn, Sn, 3 * Dn), np.float32)
    for b in range(Bn):
        seg = np.searchsorted(starts[b], t, side="right") - 1
        seg_c = np.clip(seg, 0, Sn - 1)
        in_span = (seg >= 0) & (t < ends[b][seg_c])
        valid_row = np.arange(Sn) < lengths[b]
        tok_valid = in_span & valid_row[seg_c]
        sid = np.where(tok_valid, seg_c, Sn)
        order = np.argsort(sid, kind="stable")
        ssorted = sid[order]
        xs = x[b][order]
        bounds = np.searchsorted(ssorted, np.arange(Sn + 1))
        for s in range(Sn):
            lo, hi = bounds[s], bounds[s + 1]
            if hi > lo:
                seg_x = xs[lo:hi]
                out[b, s, :Dn] = seg_x.min(axis=0)
                out[b, s, Dn : 2 * Dn] = seg_x.max(axis=0)
                out[b, s, 2 * Dn :] = seg_x.sum(axis=0, dtype=np.float32) / float(
                    hi - lo
                )
    return out


def kernel(x, lengths, span_idxs, _trace=False):
    x = np.asarray(x, dtype=np.float32)
    lengths = np.asarray(lengths, dtype=np.int32)
    span_idxs = np.asarray(span_idxs, dtype=np.int32)

    if x.shape == (B, T, D) and _is_uniform(span_idxs):
        out, exec_ns = _run_device(x, trace=_trace)
        row_ok = np.arange(S)[None, :] < lengths[:, None]
        if not row_ok.all():
            out = np.where(row_ok[..., None], out, np.float32(0.0))
        if _trace:
            return out, exec_ns
        return out

    out = _fallback(x, lengths, span_idxs)
    if _trace:
        return out, None
    return out


if __name__ == "__main__":
    rng = np.random.default_rng(0)
    x = rng.standard_normal((B, T, D), dtype=np.float32)
    starts = (np.arange(S, dtype=np.int32) * L)[None, :].repeat(B, 0)
    span_idxs = np.stack([starts, starts + L], axis=-1).astype(np.int32)
    lengths = np.full((B,), S, dtype=np.int32)
    got = kernel(x, lengths, span_idxs)
    xb = x.reshape(B, S, L, D)
    exp = np.concatenate(
        [xb.min(2), xb.max(2), xb.mean(2, dtype=np.float32)], axis=-1
    )
    err = np.abs(got - exp).max()
    print("self-test max abs err:", err, " rel:", err / np.abs(exp).max())


# revision 28
# speedup vs baseline: 1.0877x; 1.0269x over previous
"""Trainium2 Bass kernel for nn_Agg_57380763075323 (segment_reduce).

Computes, for each (batch, span): [min, max, mean] over the span's tokens of
x[B=16, T=8192, D=256], output [B, S=512, 3*D=768] float32.

Device fast path assumes the uniform span structure produced by
setup_inputs(): span s covers tokens [s*16, (s+1)*16) for all examples.
Anything else falls back to an exact numpy implementation of the reference
semantics (searchsorted-based segment assignment).

Sharding: data-parallel over batch; each of the 8 NeuronCores processes 2
examples. No cross-core communication.

Precision strategy: the output tolerance is rel_err < 2e-2; x is converted
to fp16 on the host during the shard step (rel err <= 2^-11 ~ 5e-4).  This
halves HBM load traffic (the memory-bound roofline) and doubles DVE
tensor_tensor throughput (2x_1P perf mode for 16-bit dtypes).

Device algorithm per core (2 examples, each [8192, 256] fp16), working in
PAIRS of 128-span tiles ([128, 2, 4096] fp16 = one 2MB DMA):
  - min and max via pairwise log-trees of fp16 tensor_tensor ops on the
    Vector engine; each tree level is ONE batched op across the pair
    (3D access patterns), minimizing per-op overhead (~150ns each).
    The last level emits fp16; one ScalarE copy upcasts min|max to fp32.
  - Mean via the Tensor engine: 64 normal fp16 matmuls against I/16
    accumulate transposed token chunks into one fp32 PSUM bank
    ([feat, span] layout), ScalarE copies PSUM->SBUF fp16, 4 matmuls
    against I transpose back, ScalarE copies to the fp32 result.
  - One [128, 2, 768] result tile ([min|max|mean] per span-tile) stored
    with one 768KB DMA per pair.
"""

import sys

import numpy as np

_TRN_REPO = "/opt/trn_rl_repo"

B, T, D, S = 16, 8192, 256, 512
L = T // S  # 16 tokens per span in the uniform layout
N_CORES = 8
BPC = B // N_CORES  # examples per core
P = 128  # SBUF partitions
TILES = S // P  # span-tiles per example
PAIRS = TILES // 2  # tile-pairs per example

_PROG_CACHE = {}


def _build_program():
    if _TRN_REPO not in sys.path:
        sys.path.insert(0, _TRN_REPO)
    from concourse import bacc, tile
    import concourse.mybir as mybir

    f32 = mybir.dt.float32
    f16 = mybir.dt.float16
    Alu = mybir.AluOpType

    nc = bacc.Bacc(
        "TRN2", target_bir_lowering=False, debug=False, enable_partition_id=False
    )
    x = nc.dram_tensor("x", [BPC, T, D], f16, kind="ExternalInput").ap()
    ident = nc.dram_tensor("ident", [P, 2 * P], f16, kind="ExternalInput").ap()
    # device output is fp16 (the host upcasts to fp32) — halves store traffic
    out = nc.dram_tensor("out", [BPC, S, 3 * D], f16, kind="ExternalOutput").ap()

    W = L * D  # free width per span-tile (4096)

    # [BPC, PAIRS, 128, 2, W] — partition p holds spans (2g+j)*128+p (j=0,1)
    xv = x.rearrange("b (g j p l) d -> b g p j (l d)", g=PAIRS, j=2, p=P, l=L)
    # output view matching the pair layout
    ov = out.rearrange("b (g j p) f -> b g p j f", g=PAIRS, j=2, p=P)

    with tile.TileContext(nc) as tc:
        with (
            tc.tile_pool(name="xin", bufs=3) as xin_pool,
            tc.tile_pool(name="identp", bufs=1) as ident_pool,
            tc.tile_pool(name="acc", bufs=2, space="PSUM") as acc_pool,
            tc.tile_pool(name="back", bufs=2, space="PSUM") as back_pool,
            tc.tile_pool(name="mid", bufs=2) as mid_pool,
            tc.tile_pool(name="scratch", bufs=2) as scratch,
            tc.tile_pool(name="res", bufs=2) as res_pool,
        ):
            # ident[:, 0:128] = I, ident[:, 128:256] = I/16 (both fp16)
            idt2 = ident_pool.tile([P, 2 * P], f16)
            nc.scalar.dma_start(out=idt2, in_=ident)
            idt = idt2[:, 0:P]
            idtS = idt2[:, P : 2 * P]

            for b in range(BPC):
                for g in range(PAIRS):
                    first = b == 0 and g == 0
                    last = b == BPC - 1 and g == PAIRS - 1
                    t = xin_pool.tile([P, 2, W], f16, tag="xin")
                    if first:
                        # warmup: progressive chunks so the DVE starts as
                        # soon as the first 64KB lands
                        bounds0 = [0, 1024, 2048, 3072, W]
                        for lo, hi in zip(bounds0[:-1], bounds0[1:]):
                            nc.sync.dma_start(
                                out=t[:, 0, lo:hi], in_=xv[b, g][:, 0, lo:hi])
                        for q in range(2):
                            CW = W // 2
                            nc.sync.dma_start(
                                out=t[:, 1, q * CW : (q + 1) * CW],
                                in_=xv[b, g][:, 1, q * CW : (q + 1) * CW])
                    else:
                        nc.sync.dma_start(out=t, in_=xv[b, g])

                    res2 = res_pool.tile([P, 2, 3 * D], f16, tag="res")

                    # --- min/max trees (DVE) ---
                    s1n = scratch.tile([P, 2, W // 2], f16, tag="s1n")
                    s1x = scratch.tile([P, 2, W // 2], f16, tag="s1x")
                    HW_ = W // 2
                    if first:
                        # L1 gated per DMA chunk (pairs tokens within each
                        # chunk — valid since min/max are commutative)
                        pieces = [(0, lo, hi) for lo, hi in
                                  zip(bounds0[:-1], bounds0[1:])]
                        pieces += [(1, 0, W // 2), (1, W // 2, W)]
                        for j, lo, hi in pieces:
                            E = (hi - lo) // 2
                            for s1, op in ((s1n, Alu.min), (s1x, Alu.max)):
                                nc.vector.tensor_tensor(
                                    out=s1[:, j, lo // 2 : hi // 2],
                                    in0=t[:, j, lo : lo + E],
                                    in1=t[:, j, lo + E : hi], op=op)
                    elif last:
                        # last pair: per-j trees so sub-results store early
                        for j in range(2):
                            for s1, op in ((s1n, Alu.min), (s1x, Alu.max)):
                                nc.vector.tensor_tensor(
                                    out=s1[:, j, :], in0=t[:, j, 0:HW_],
                                    in1=t[:, j, HW_:W], op=op)
                    else:
                        for s1, op in ((s1n, Alu.min), (s1x, Alu.max)):
                            nc.vector.tensor_tensor(
                                out=s1, in0=t[:, :, 0:HW_],
                                in1=t[:, :, HW_:W], op=op)

                    s2n = scratch.tile([P, 2, W // 4], f16, tag="s2n")
                    s2x = scratch.tile([P, 2, W // 4], f16, tag="s2x")
                    s3n = scratch.tile([P, 2, W // 8], f16, tag="s3n")
                    s3x = scratch.tile([P, 2, W // 8], f16, tag="s3x")

                    def levels(js):
                        """L2..L4 over j-slice js (slice or full), writing
                        res2[:, js, 0:2D]."""
                        for sa, sb, op in ((s1n, s2n, Alu.min),
                                           (s1x, s2x, Alu.max)):
                            nc.vector.tensor_tensor(
                                out=sb[:, js, :], in0=sa[:, js, 0 : W // 4],
                                in1=sa[:, js, W // 4 : W // 2], op=op)
                        for sa, sb, op in ((s2n, s3n, Alu.min),
                                           (s2x, s3x, Alu.max)):
                            nc.vector.tensor_tensor(
                                out=sb[:, js, :], in0=sa[:, js, 0 : W // 8],
                                in1=sa[:, js, W // 8 : W // 4], op=op)
                        nc.vector.tensor_tensor(
                            out=res2[:, js, 0:D], in0=s3n[:, js, 0:D],
                            in1=s3n[:, js, D : 2 * D], op=Alu.min)
                        nc.vector.tensor_tensor(
                            out=res2[:, js, D : 2 * D], in0=s3x[:, js, 0:D],
                            in1=s3x[:, js, D : 2 * D], op=Alu.max)

                    if last:
                        levels(slice(0, 1))
                        nc.scalar.dma_start(
                            out=ov[b, g][:, 0:1, 0 : 2 * D],
                            in_=res2[:, 0:1, 0 : 2 * D])
                        levels(slice(1, 2))
                        nc.scalar.dma_start(
                            out=ov[b, g][:, 1:2, 0 : 2 * D],
                            in_=res2[:, 1:2, 0 : 2 * D])
                    else:
                        levels(slice(None))

                    # --- mean via PE ---
                    acc = acc_pool.tile([P, 4 * P], f32, tag="acc")
                    for j in range(2):
                        for h in range(2):
                            gidx = j * 2 + h
                            for tok in range(L):
                                c = 2 * tok + h
                                nc.tensor.matmul(
                                    out=acc[:, gidx * P : (gidx + 1) * P],
                                    lhsT=t[:, j, c * P : (c + 1) * P],
                                    rhs=idtS,
                                    start=(tok == 0),
                                    stop=(tok == L - 1),
                                )
                    mid = mid_pool.tile([P, 4 * P], f16, tag="mid")
                    nc.scalar.copy(out=mid, in_=acc)
                    backp = back_pool.tile([P, 4 * P], f32, tag="back")
                    for gidx in range(4):
                        nc.tensor.matmul(
                            out=backp[:, gidx * P : (gidx + 1) * P],
                            lhsT=mid[:, gidx * P : (gidx + 1) * P],
                            rhs=idt,
                            start=True,
                            stop=True,
                        )
                    # backp columns are ordered (j, h, d) == res2[:, j, 2D:3D]
                    nc.scalar.copy(out=res2[:, :, 2 * D : 3 * D], in_=backp)

                    # --- store (mean separate: it's ready before the trees) ---
                    nc.scalar.dma_start(
                        out=ov[b, g][:, :, 2 * D : 3 * D],
                        in_=res2[:, :, 2 * D : 3 * D])
                    if not last:
                        nc.scalar.dma_start(
                            out=ov[b, g][:, :, 0 : 2 * D],
                            in_=res2[:, :, 0 : 2 * D])
    nc.compile()
    return nc


def _get_program():
    if "nc" not in _PROG_CACHE:
        _PROG_CACHE["nc"] = _build_program()
    return _PROG_CACHE["nc"]


def _ensure_ntff_hook():
    """Register the axon NTFF profiling hook if the image lacks
    antenv.axon_hooks (replicates trn_boot._ntff_profile_via_ctypes)."""
    try:
        from antenv.axon_hooks import get_axon_ntff_profile_hook  # noqa: F401

        return
    except ImportError:
        pass
    import contextlib
    import ctypes
    import types

    try:
        import antenv
    except ImportError:
        return

    so_path = "/opt/axon/libaxon_pjrt.so"
    mod = types.ModuleType("antenv.axon_hooks")
    holder = {"hook": None}
    mod.set_axon_ntff_profile_hook = lambda h: holder.__setitem__("hook", h)
    mod.get_axon_ntff_profile_hook = lambda: holder["hook"]
    sys.modules["antenv.axon_hooks"] = mod
    antenv.axon_hooks = mod

    try:
        lib = ctypes.CDLL(so_path)
    except OSError:
        return
    if not hasattr(lib, "axon_start_nrt_profile"):
        return
    lib.axon_start_nrt_profile.argtypes = [
        ctypes.POINTER(ctypes.c_int64),
        ctypes.c_size_t,
    ]
    lib.axon_start_nrt_profile.restype = ctypes.c_int64
    lib.axon_stop_nrt_profile.argtypes = [ctypes.c_char_p]
    lib.axon_stop_nrt_profile.restype = ctypes.c_int64

    @contextlib.contextmanager
    def _hook(output_dir, device_ids):
        import jax

        jax.devices()
        if device_ids:
            ids = (ctypes.c_int64 * len(device_ids))(*device_ids)
            rc = lib.axon_start_nrt_profile(ids, len(device_ids))
        else:
            rc = lib.axon_start_nrt_profile(None, 0)
        if rc != 0:
            raise RuntimeError(f"axon_start_nrt_profile rc={rc}")
        try:
            yield
        finally:
            n = lib.axon_stop_nrt_profile(str(output_dir).encode())
            if n < 0:
                raise RuntimeError(f"axon_stop_nrt_profile rc={n}")
            if n == 0:
                print(f"profile: 0 files written to {output_dir}", file=sys.stderr)

    mod.set_axon_ntff_profile_hook(_hook)


def _run_device(x, trace=False):
    """x: [B, T, D] float32 (uniform span layout). Returns ([B, S, 3D], exec_ns)."""
    if _TRN_REPO not in sys.path:
        sys.path.insert(0, _TRN_REPO)
    if trace:
        _ensure_ntff_hook()
    from concourse.bass_utils import run_bass_kernel_spmd

    nc = _get_program()
    x16 = x.astype(np.float16)
    eye = np.eye(P, dtype=np.float16)
    ident = np.concatenate([eye, eye / np.float16(L)], axis=1)
    in_maps = [
        {"x": np.ascontiguousarray(x16[c * BPC : (c + 1) * BPC]), "ident": ident}
        for c in range(N_CORES)
    ]
    res = run_bass_kernel_spmd(
        nc, in_maps, core_ids=list(range(N_CORES)), trace=trace
    )
    out = np.concatenate(
        [res.results[c]["out"] for c in range(N_CORES)], axis=0
    ).astype(np.float32)
    # Output order per row is [min | max | mean]; reference order is
    # [smin, smax, mean] — identical.
    return out, res.exec_time_ns


def _is_uniform(span_idxs):
    if span_idxs.shape != (B, S, 2):
        return False
    starts = np.arange(S, dtype=np.int64) * L
    return bool(
        np.all(span_idxs[..., 0] == starts[None, :])
        and np.all(span_idxs[..., 1] == starts[None, :] + L)
    )


def _fallback(x, lengths, span_idxs):
    """Exact numpy port of the reference semantics (general spans)."""
    Bn, Tn, Dn = x.shape
    Sn = span_idxs.shape[1]
    starts = span_idxs[..., 0]
    ends = span_idxs[..., 1]
    t = np.arange(Tn)
    out = np.zeros((Bn, Sn, 3 * Dn), np.float32)
    for b in range(Bn):
        seg = np.searchsorted(starts[b], t, side="right") - 1
        seg_c = np.clip(seg, 0, Sn - 1)
        in_span = (seg >= 0) & (t < ends[b][seg_c])
        valid_row = np.arange(Sn) < lengths[b]
        tok_valid = in_span & valid_row[seg_c]
        sid = np.where(tok_valid, seg_c, Sn)
        order = np.argsort(sid, kind="stable")
        ssorted = sid[order]
        xs = x[b][order]
        bounds = np.searchsorted(ssorted, np.arange(Sn + 1))
        for s in range(Sn):
            lo, hi = bounds[s], bounds[s + 1]
            if hi > lo:
                seg_x = xs[lo:hi]
                out[b, s, :Dn] = seg_x.min(axis=0)
                out[b, s, Dn : 2 * Dn] = seg_x.max(axis=0)
                out[b, s, 2 * Dn :] = seg_x.sum(axis=0, dtype=np.float32) / float(
                    hi - lo
                )
    return out


def kernel(x, lengths, span_idxs, _trace=False):
    x = np.asarray(x, dtype=np.float32)
    lengths = np.asarray(lengths, dtype=np.int32)
    span_idxs = np.asarray(span_idxs, dtype=np.int32)

    if x.shape == (B, T, D) and _is_uniform(span_idxs):
        out, exec_ns = _run_device(x, trace=_trace)
        row_ok = np.arange(S)[None, :] < lengths[:, None]
        if not row_ok.all():
            out = np.where(row_ok[..., None], out, np.float32(0.0))
        if _trace:
            return out, exec_ns
        return out

    out = _fallback(x, lengths, span_idxs)
    if _trace:
        return out, None
    return out


if __name__ == "__main__":
    rng = np.random.default_rng(0)
    x = rng.standard_normal((B, T, D), dtype=np.float32)
    starts = (np.arange(S, dtype=np.int32) * L)[None, :].repeat(B, 0)
    span_idxs = np.stack([starts, starts + L], axis=-1).astype(np.int32)
    lengths = np.full((B,), S, dtype=np.int32)
    got = kernel(x, lengths, span_idxs)
    xb = x.reshape(B, S, L, D)
    exp = np.concatenate(
        [xb.min(2), xb.max(2), xb.mean(2, dtype=np.float32)], axis=-1
    )
    err = np.abs(got - exp).max()
    print("self-test max abs err:", err, " rel:", err / np.abs(exp).max())
